# revision 22
# baseline (speedup 1.0000x reference)
"""GCN layer kernel for Trainium2, SPMD over 8 NeuronCores.

Reference computation (all fp32):
    adj_hat = rownorm(adj + I)                      # [N, N]
    out     = adj_hat @ (X @ W) + bias              # X: [N, T, A]

Sharding: T (time) axis split across 8 cores; adj/W/bias replicated.

Per-core kernel (T_SH = 256 time steps):
  setup (once): load adj [m,n], PE-transpose to adjT [n,m], add I on the
    diagonal blocks, column-normalize via a ones-vector matmul + reciprocal
    + partition_broadcast; load W; build a partition-broadcast bias tile.
  per t: Y_t^T[a, m] = sum_nck matmul(lhsT=X_t[n,a], rhs=adjT_hat[n,m])
         (X's natural [n, (t a)] SBUF layout is exactly the stationary
         operand - no transposes anywhere in the hot loop)
         out_t[m, o] = matmul(lhsT=Y_t^T[a, m_half], rhs=W[a, o])  x2
         out_sb = out_psum + bias_bcast  (fused with the PSUM->SBUF copy)
"""

import os
import sys

import numpy as np

for _p in ("/opt/trn_rl_repo", "/root/.axon_site/_ro/trn_rl_repo"):
    if os.path.isdir(_p) and _p not in sys.path:
        sys.path.insert(0, _p)

import concourse.bass as bass
import concourse.mybir as mybir
import concourse.tile as tile
from concourse import bacc
from concourse.bass_utils import run_bass_kernel_spmd
from concourse.masks import make_identity

N_NODES = 256
N_TIMES = 2048
N_FEAT = 128
N_CORES = 8
T_SH = N_TIMES // N_CORES  # 256 time steps per core
P = 128  # partitions
NCH = N_NODES // P  # 2 node chunks

F32 = mybir.dt.float32


def _gcn_body(tc, out, x, adj, w, b, t_sh, tb, g1_f32r=True, g2_f32r=False):
    nc = tc.nc
    nblk = t_sh // tb
    F32R = mybir.dt.float32r
    # fp32r (fp32 truncated to 11 mantissa bits) streams at 1 cycle/col for
    # N>=256 vs fp32's 4 passes. GEMM1 (N=256) uses it; GEMM2 (N=128) stays
    # full fp32 - it is hidden under the DMA roofline anyway.
    g1_dt = F32R if g1_f32r else F32
    g2_dt = F32R if g2_f32r else F32
    g2c = (lambda ap: ap.bitcast(F32R)) if g2_f32r else (lambda ap: ap)

    from contextlib import ExitStack

    with ExitStack() as ctx:
        const = ctx.enter_context(tc.tile_pool(name="const", bufs=1))

        ident = const.tile([P, P], F32)
        make_identity(nc, ident)

        w_sb = const.tile([P, P], F32)
        nc.sync.dma_start(out=w_sb, in_=w)

        # bias replicated across all 128 partitions (free dim = output feature)
        bias_bc = const.tile([P, N_FEAT], F32)
        bias_bcast_ap = bass.AP(
            tensor=b.tensor, offset=b.offset, ap=[[0, P], b.ap[0]]
        )
        nc.sync.dma_start(out=bias_bc, in_=bias_bcast_ap)

        # adjT_hat[n, m] = (adj[m, n] + I) / deg[m], n on partitions
        adjT = [
            const.tile([P, N_NODES], g1_dt, name=f"adjT{c}", tag=f"adjT{c}")
            for c in range(NCH)
        ]

        # Main-loop SBUF pools are created BEFORE the setup scratch pool so
        # their addresses don't alias it - otherwise the first X-tile DMAs
        # inherit a WAR dependency on the whole adjacency-setup chain and the
        # DMA queue sits idle for ~20us at kernel start.
        xp = ctx.enter_context(tc.tile_pool(name="xp", bufs=3))
        op = ctx.enter_context(tc.tile_pool(name="op", bufs=3))
        ysb = ctx.enter_context(tc.tile_pool(name="ysb", bufs=tb + 2))

        def load_x(blk):
            t0 = blk * tb
            xt = []
            for ck in range(NCH):
                xtc = xp.tile(
                    [P, tb, N_FEAT], g1_dt, name=f"x{ck}_{blk}", tag=f"x{ck}"
                )
                nc.sync.dma_start(
                    out=xtc, in_=x[ck * P : (ck + 1) * P, t0 : t0 + tb, :]
                )
                xt.append(xtc)
            return xt

        PF = 2  # blocks prefetched ahead of the setup phase
        prefetched = [load_x(blk) for blk in range(min(PF, nblk))]

        setup = ctx.enter_context(tc.tile_pool(name="setup", bufs=1))
        with tc.tile_pool(name="setup_ps", bufs=1, space="PSUM") as setup_ps:
            a_sb = []
            for mc in range(NCH):
                a_t = setup.tile([P, N_NODES], F32, name=f"a{mc}", tag=f"a{mc}")
                nc.sync.dma_start(out=a_t, in_=adj[mc * P : (mc + 1) * P, :])
                a_sb.append(a_t)
            for nck in range(NCH):
                for mc in range(NCH):
                    tp = setup_ps.tile([P, P], F32, name="tp", tag="tp")
                    nc.tensor.transpose(
                        tp, a_sb[mc][:, nck * P : (nck + 1) * P], ident
                    )
                    dst = adjT[nck][:, mc * P : (mc + 1) * P]
                    if mc == nck:
                        nc.vector.tensor_add(dst, tp, ident)
                    else:
                        nc.vector.tensor_copy(dst, tp)
            # deg[m] = sum_n adjT[n, m] (self-loop already included)
            ones_f32 = setup.tile([P, 1], F32)
            nc.vector.memset(ones_f32, 1.0)
            ones_col = setup.tile([P, 1], g1_dt)
            nc.vector.tensor_copy(ones_col, ones_f32)
            deg_ps = setup_ps.tile([1, N_NODES], F32, name="deg", tag="deg")
            for nck in range(NCH):
                nc.tensor.matmul(
                    deg_ps,
                    ones_col,
                    adjT[nck],
                    start=(nck == 0),
                    stop=(nck == NCH - 1),
                )
            rdeg = setup.tile([1, N_NODES], F32)
            nc.vector.reciprocal(rdeg, deg_ps)
            rdeg_bc = setup.tile([P, N_NODES], F32)
            nc.gpsimd.partition_broadcast(rdeg_bc, rdeg)
            for nck in range(NCH):
                nc.vector.tensor_mul(adjT[nck], adjT[nck], rdeg_bc)

        yps = ctx.enter_context(tc.tile_pool(name="yps", bufs=3, space="PSUM"))
        ops = ctx.enter_context(tc.tile_pool(name="ops", bufs=2, space="PSUM"))

        for blk in range(nblk):
            t0 = blk * tb
            xt = prefetched[blk] if blk < len(prefetched) else load_x(blk)
            ot = [
                op.tile([P, tb, N_FEAT], F32, name=f"o{mc}_{blk}", tag=f"o{mc}")
                for mc in range(NCH)
            ]
            # Phase 1: all aggregation matmuls of the block + PSUM->SBUF
            # copies (ACT). Keeping PE on back-to-back GEMM1s gives the
            # copies time to land before phase 2 consumes them, so the
            # in-order PE queue never stalls on the DVE/ACT engines.
            ys_list = []
            for ti in range(tb):
                ypt = yps.tile([P, N_NODES], F32, name="ypt", tag="y")
                for ck in range(NCH):
                    nc.tensor.matmul(
                        ypt,
                        xt[ck][:, ti, :],
                        adjT[ck],
                        start=(ck == 0),
                        stop=(ck == NCH - 1),
                    )
                ys = ysb.tile([P, N_NODES], F32, name=f"ys{ti}", tag="ys")
                nc.scalar.copy(ys, ypt)
                ys_list.append(ys)
            # Phase 2: feature-transform matmuls + bias add (DVE)
            for ti in range(tb):
                for mc in range(NCH):
                    opt = ops.tile([P, N_FEAT], F32, name="opt", tag=f"op{mc}")
                    nc.tensor.matmul(
                        opt,
                        g2c(ys_list[ti][:, mc * P : (mc + 1) * P]),
                        g2c(w_sb),
                        start=True,
                        stop=True,
                    )
                    nc.vector.tensor_add(ot[mc][:, ti, :], opt, bias_bc)
            for mc in range(NCH):
                nc.sync.dma_start(
                    out=out[mc * P : (mc + 1) * P, t0 : t0 + tb, :], in_=ot[mc]
                )


def build(t_sh=T_SH, tb=16, g1_f32r=True, g2_f32r=False):
    """Build + compile the per-core Bass module."""
    nc = bacc.Bacc(
        "TRN2", target_bir_lowering=False, debug=False, num_devices=N_CORES
    )
    x_dt = mybir.dt.float32r if g1_f32r else F32
    x = nc.dram_tensor("node_feats", [N_NODES, t_sh, N_FEAT], x_dt, kind="ExternalInput").ap()
    adj = nc.dram_tensor("adj_matrix", [N_NODES, N_NODES], F32, kind="ExternalInput").ap()
    w = nc.dram_tensor("weight", [N_FEAT, N_FEAT], F32, kind="ExternalInput").ap()
    b = nc.dram_tensor("bias", [N_FEAT], F32, kind="ExternalInput").ap()
    out = nc.dram_tensor("out", [N_NODES, t_sh, N_FEAT], F32, kind="ExternalOutput").ap()
    with tile.TileContext(nc) as tc:
        _gcn_body(tc, out, x, adj, w, b, t_sh, tb, g1_f32r=g1_f32r, g2_f32r=g2_f32r)
    nc.compile()
    return nc


_built_nc = None


def _get_nc():
    global _built_nc
    if _built_nc is None:
        _built_nc = build()
    return _built_nc


def _run(node_feats, adj_matrix, weight, bias, trace=False, tmpdir=None):
    nc = _get_nc()
    node_feats = np.ascontiguousarray(node_feats, dtype=np.float32)
    adj_matrix = np.ascontiguousarray(adj_matrix, dtype=np.float32)
    weight = np.ascontiguousarray(weight, dtype=np.float32)
    bias = np.ascontiguousarray(bias, dtype=np.float32)
    in_maps = [
        {
            "node_feats": np.ascontiguousarray(
                node_feats[:, c * T_SH : (c + 1) * T_SH, :]
            ),
            "adj_matrix": adj_matrix,
            "weight": weight,
            "bias": bias,
        }
        for c in range(N_CORES)
    ]
    res = run_bass_kernel_spmd(
        nc, in_maps, list(range(N_CORES)), trace=trace, tmpdir=tmpdir
    )
    out = np.concatenate(
        [res.results[c]["out"] for c in range(N_CORES)], axis=1
    )
    return out, res


def kernel(node_feats, adj_matrix, weight, bias):
    out, _ = _run(node_feats, adj_matrix, weight, bias)
    return out


# revision 25
# speedup vs baseline: 1.0267x; 1.0267x over previous
"""GCN layer kernel for Trainium2, SPMD over 8 NeuronCores.

Reference computation (all fp32):
    adj_hat = rownorm(adj + I)                      # [N, N]
    out     = adj_hat @ (X @ W) + bias              # X: [N, T, A]

Sharding: T (time) axis split across 8 cores; adj/W/bias replicated.

Per-core kernel (T_SH = 256 time steps):
  setup (once): load adj [m,n], PE-transpose to adjT [n,m], add I on the
    diagonal blocks, column-normalize via a ones-vector matmul + reciprocal
    + partition_broadcast; load W; build a partition-broadcast bias tile.
  per t: Y_t^T[a, m] = sum_nck matmul(lhsT=X_t[n,a], rhs=adjT_hat[n,m])
         (X's natural [n, (t a)] SBUF layout is exactly the stationary
         operand - no transposes anywhere in the hot loop)
         out_t[m, o] = matmul(lhsT=Y_t^T[a, m_half], rhs=W[a, o])  x2
         out_sb = out_psum + bias_bcast  (fused with the PSUM->SBUF copy)
"""

import os
import sys

import numpy as np

for _p in ("/opt/trn_rl_repo", "/root/.axon_site/_ro/trn_rl_repo"):
    if os.path.isdir(_p) and _p not in sys.path:
        sys.path.insert(0, _p)

import concourse.bass as bass
import concourse.mybir as mybir
import concourse.tile as tile
from concourse import bacc
from concourse.bass_utils import run_bass_kernel_spmd
from concourse.masks import make_identity

N_NODES = 256
N_TIMES = 2048
N_FEAT = 128
N_CORES = 8
T_SH = N_TIMES // N_CORES  # 256 time steps per core
P = 128  # partitions
NCH = N_NODES // P  # 2 node chunks

F32 = mybir.dt.float32


def _gcn_body(tc, out, x, adj, w, b, t_sh, tb, g1_f32r=True, g2_f32r=False):
    nc = tc.nc
    nblk = t_sh // tb
    F32R = mybir.dt.float32r
    # fp32r (fp32 truncated to 11 mantissa bits) streams at 1 cycle/col for
    # N>=256 vs fp32's 4 passes. GEMM1 (N=256) uses it; GEMM2 (N=128) stays
    # full fp32 - it is hidden under the DMA roofline anyway.
    g1_dt = F32R if g1_f32r else F32
    g2_dt = F32R if g2_f32r else F32
    g2c = (lambda ap: ap.bitcast(F32R)) if g2_f32r else (lambda ap: ap)

    from contextlib import ExitStack

    with ExitStack() as ctx:
        const = ctx.enter_context(tc.tile_pool(name="const", bufs=1))

        ident = const.tile([P, P], F32)
        make_identity(nc, ident)

        w_sb = const.tile([P, P], F32)
        nc.sync.dma_start(out=w_sb, in_=w)

        # bias replicated across all 128 partitions (free dim = output feature)
        bias_bc = const.tile([P, N_FEAT], F32)
        bias_bcast_ap = bass.AP(
            tensor=b.tensor, offset=b.offset, ap=[[0, P], b.ap[0]]
        )
        nc.sync.dma_start(out=bias_bc, in_=bias_bcast_ap)

        # adjT_hat[n, m] = (adj[m, n] + I) / deg[m], n on partitions
        adjT = [
            const.tile([P, N_NODES], g1_dt, name=f"adjT{c}", tag=f"adjT{c}")
            for c in range(NCH)
        ]

        # Main-loop SBUF pools are created BEFORE the setup scratch pool so
        # their addresses don't alias it - otherwise the first X-tile DMAs
        # inherit a WAR dependency on the whole adjacency-setup chain and the
        # DMA queue sits idle for ~20us at kernel start.
        xp = ctx.enter_context(tc.tile_pool(name="xp", bufs=4))
        op = ctx.enter_context(tc.tile_pool(name="op", bufs=3))
        ysb = ctx.enter_context(tc.tile_pool(name="ysb", bufs=tb + 2))

        def load_x(blk):
            t0 = blk * tb
            xt = []
            for ck in range(NCH):
                xtc = xp.tile(
                    [P, tb, N_FEAT], g1_dt, name=f"x{ck}_{blk}", tag=f"x{ck}"
                )
                nc.sync.dma_start(
                    out=xtc, in_=x[ck * P : (ck + 1) * P, t0 : t0 + tb, :]
                )
                xt.append(xtc)
            return xt

        PF = 2  # blocks prefetched ahead of the setup phase
        prefetched = [load_x(blk) for blk in range(min(PF, nblk))]

        # adjT holds the UNnormalized (adj + I)^T; the 1/deg row scaling is
        # applied at the very end as a per-partition scalar, so GEMM1 only
        # waits on the 4 PE transposes (short setup critical path).
        r_m = [
            const.tile([P, 1], F32, name=f"r{mc}", tag=f"r{mc}")
            for mc in range(NCH)
        ]
        setup = ctx.enter_context(tc.tile_pool(name="setup", bufs=1))
        with tc.tile_pool(name="setup_ps", bufs=1, space="PSUM") as setup_ps:
            a_sb = []
            for mc in range(NCH):
                a_t = setup.tile([P, N_NODES], F32, name=f"a{mc}", tag=f"a{mc}")
                nc.sync.dma_start(out=a_t, in_=adj[mc * P : (mc + 1) * P, :])
                a_sb.append(a_t)
            for nck in range(NCH):
                for mc in range(NCH):
                    tp = setup_ps.tile([P, P], F32, name="tp", tag="tp")
                    nc.tensor.transpose(
                        tp, a_sb[mc][:, nck * P : (nck + 1) * P], ident
                    )
                    dst = adjT[nck][:, mc * P : (mc + 1) * P]
                    if mc == nck:
                        nc.vector.tensor_add(dst, tp, ident)
                    else:
                        nc.vector.tensor_copy(dst, tp)
            # r[m] = 1 / (1 + sum_n adj[m, n]) straight off the natural
            # [m, n] layout - no transpose or broadcast needed.
            for mc in range(NCH):
                dg = setup.tile([P, 1], F32, name=f"dg{mc}", tag=f"dg{mc}")
                nc.vector.reduce_sum(dg, a_sb[mc], axis=mybir.AxisListType.X)
                nc.vector.tensor_scalar_add(dg, dg, 1.0)
                nc.vector.reciprocal(r_m[mc], dg)

        yps = ctx.enter_context(tc.tile_pool(name="yps", bufs=3, space="PSUM"))
        ops = ctx.enter_context(tc.tile_pool(name="ops", bufs=2, space="PSUM"))

        for blk in range(nblk):
            t0 = blk * tb
            xt = prefetched[blk] if blk < len(prefetched) else load_x(blk)
            ot = [
                op.tile([P, tb, N_FEAT], F32, name=f"o{mc}_{blk}", tag=f"o{mc}")
                for mc in range(NCH)
            ]
            # Phase 1: all aggregation matmuls of the block + PSUM->SBUF
            # copies (ACT). Keeping PE on back-to-back GEMM1s gives the
            # copies time to land before phase 2 consumes them, so the
            # in-order PE queue never stalls on the DVE/ACT engines.
            ys_list = []
            for ti in range(tb):
                ypt = yps.tile([P, N_NODES], F32, name="ypt", tag="y")
                for ck in range(NCH):
                    nc.tensor.matmul(
                        ypt,
                        xt[ck][:, ti, :],
                        adjT[ck],
                        start=(ck == 0),
                        stop=(ck == NCH - 1),
                    )
                ys = ysb.tile([P, N_NODES], F32, name=f"ys{ti}", tag="ys")
                nc.scalar.copy(ys, ypt)
                ys_list.append(ys)
            # Phase 2: feature-transform matmuls + bias add (DVE)
            for ti in range(tb):
                for mc in range(NCH):
                    opt = ops.tile([P, N_FEAT], F32, name="opt", tag=f"op{mc}")
                    nc.tensor.matmul(
                        opt,
                        g2c(ys_list[ti][:, mc * P : (mc + 1) * P]),
                        g2c(w_sb),
                        start=True,
                        stop=True,
                    )
                    nc.vector.scalar_tensor_tensor(
                        out=ot[mc][:, ti, :],
                        in0=opt,
                        scalar=r_m[mc],
                        in1=bias_bc,
                        op0=mybir.AluOpType.mult,
                        op1=mybir.AluOpType.add,
                    )
            for mc in range(NCH):
                nc.sync.dma_start(
                    out=out[mc * P : (mc + 1) * P, t0 : t0 + tb, :], in_=ot[mc]
                )


def build(t_sh=T_SH, tb=16, g1_f32r=True, g2_f32r=False):
    """Build + compile the per-core Bass module."""
    nc = bacc.Bacc(
        "TRN2", target_bir_lowering=False, debug=False, num_devices=N_CORES
    )
    x_dt = mybir.dt.float32r if g1_f32r else F32
    x = nc.dram_tensor("node_feats", [N_NODES, t_sh, N_FEAT], x_dt, kind="ExternalInput").ap()
    adj = nc.dram_tensor("adj_matrix", [N_NODES, N_NODES], F32, kind="ExternalInput").ap()
    w = nc.dram_tensor("weight", [N_FEAT, N_FEAT], F32, kind="ExternalInput").ap()
    b = nc.dram_tensor("bias", [N_FEAT], F32, kind="ExternalInput").ap()
    out = nc.dram_tensor("out", [N_NODES, t_sh, N_FEAT], F32, kind="ExternalOutput").ap()
    with tile.TileContext(nc) as tc:
        _gcn_body(tc, out, x, adj, w, b, t_sh, tb, g1_f32r=g1_f32r, g2_f32r=g2_f32r)
    nc.compile()
    return nc


_built_nc = None


def _get_nc():
    global _built_nc
    if _built_nc is None:
        _built_nc = build()
    return _built_nc


def _run(node_feats, adj_matrix, weight, bias, trace=False, tmpdir=None):
    nc = _get_nc()
    node_feats = np.ascontiguousarray(node_feats, dtype=np.float32)
    adj_matrix = np.ascontiguousarray(adj_matrix, dtype=np.float32)
    weight = np.ascontiguousarray(weight, dtype=np.float32)
    bias = np.ascontiguousarray(bias, dtype=np.float32)
    in_maps = [
        {
            "node_feats": np.ascontiguousarray(
                node_feats[:, c * T_SH : (c + 1) * T_SH, :]
            ),
            "adj_matrix": adj_matrix,
            "weight": weight,
            "bias": bias,
        }
        for c in range(N_CORES)
    ]
    res = run_bass_kernel_spmd(
        nc, in_maps, list(range(N_CORES)), trace=trace, tmpdir=tmpdir
    )
    out = np.concatenate(
        [res.results[c]["out"] for c in range(N_CORES)], axis=1
    )
    return out, res


def kernel(node_feats, adj_matrix, weight, bias):
    out, _ = _run(node_feats, adj_matrix, weight, bias)
    return out
